# revision 9
# baseline (speedup 1.0000x reference)
"""Multi-head attention (B=8, N=1024, C=768, H=12) on 8 TRN2 NeuronCores.

Sharding: data-parallel over the batch — core i computes batch element i.
No collectives.

Per-core math (all feature-major to avoid on-device transposes):
  qkT   = w_qkv[:, :1536].T @ xT            # [1536, 1024]  (q rows 0:768, k rows 768:1536)
  v_tok = xT.T @ w_qkv[:, 1536:]            # [1024, 768]   token-major, + ones col per head
  per head h:
    ST   = kT_h.T(as lhsT) ... = k @ q^T    # [1024k, 1024q]  (scores transposed)
    E    = exp(SCALE * ST)                  # bf16, no max-subtraction (scores ~ N(0,1))
    [O_un; d] = [v_h | 1].T @ E             # [65, 1024q]: rows 0:64 = (P@V)^T un-normalized,
                                            #              row 64 = softmax denominator
    OT_h = O_un * (1/d broadcast)           # broadcast via K=1 outer-product matmul
  yT = w_proj.T @ OT + b_proj               # [768, 1024]

Host side: kernel() takes full inputs, pre-transposes x, runs SPMD on 8 cores,
re-transposes/stacks the per-core outputs.
"""

import os
import sys

import numpy as np

for _p in ("/opt/trn_rl_repo", "/root/.axon_site/_ro/trn_rl_repo"):
    if os.path.isdir(_p) and _p not in sys.path:
        sys.path.insert(0, _p)

import concourse.bacc as bacc
import concourse.mybir as mybir
import concourse.tile as tile

F32 = mybir.dt.float32
F32R = mybir.dt.float32r
BF16 = mybir.dt.bfloat16

B, NT, C = 8, 1024, 768
H, HD = 12, 64
C3 = 3 * C          # 2304
SCALE = HD ** -0.5  # 0.125
KT = C // 128       # 6   k-tiles over the C contraction
MQK = 1536 // 128   # 12  row-blocks of qkT
TT = NT // 128      # 8   token tiles
NQ = NT // 512      # 2   512-wide q slices
VA = HD + 1         # 65  v columns per head + ones column


def build_graph(tc):
    nc = tc.nc
    xt_d = nc.dram_tensor("xT", [C, NT], F32R, kind="ExternalInput").ap()
    wqkv_d = nc.dram_tensor("wqkv", [C, C3], F32R, kind="ExternalInput").ap()
    wproj_d = nc.dram_tensor("wproj", [C, C], F32R, kind="ExternalInput").ap()
    bproj_d = nc.dram_tensor("bproj", [128, KT], F32, kind="ExternalInput").ap()
    out_d = nc.dram_tensor("out", [C, NT], F32, kind="ExternalOutput").ap()

    from contextlib import ExitStack

    with ExitStack() as stack:
        persist = stack.enter_context(tc.tile_pool(name="persist", bufs=1))
        qk_sb = persist.tile([128, MQK * NT], F32R)       # qkT feature-major
        vaug = persist.tile([128, TT * H * VA], BF16)    # [v_h | 1] per head, token-major
        ot = persist.tile([128, KT * NT], F32R)           # attention out, feature-major

        ps_main = stack.enter_context(
            tc.tile_pool(name="ps_main", bufs=4, space="PSUM")
        )
        ps_pv = stack.enter_context(tc.tile_pool(name="ps_pv", bufs=4, space="PSUM"))

        nc.vector.memset(vaug[:, :], 1.0)

        # ---------------- phase 1: qkT and v_tok ----------------
        with tc.tile_pool(name="ph1", bufs=1) as ph1:
            wq_sb = ph1.tile([128, KT * C3], F32R)
            xt_sb = ph1.tile([128, KT * NT], F32R)

            for k in range(KT):
                nc.sync.dma_start(
                    out=xt_sb[:, k * NT : (k + 1) * NT],
                    in_=xt_d[k * 128 : (k + 1) * 128, :],
                )
                # v columns first so v_tok matmuls can start early
                nc.sync.dma_start(
                    out=wq_sb[:, k * C3 + 1536 : k * C3 + C3],
                    in_=wqkv_d[k * 128 : (k + 1) * 128, 1536:C3],
                )
            for k in range(KT):
                nc.sync.dma_start(
                    out=wq_sb[:, k * C3 : k * C3 + 1536],
                    in_=wqkv_d[k * 128 : (k + 1) * 128, 0:1536],
                )

            # v_tok: [1024 tok, 768] = xT.T @ w_qkv[:, 1536:2304], evicted into
            # vaug (bf16) with a ones column per head
            for t in range(TT):
                for j in range(2):  # 384-wide v-col slices → heads 6j..6j+5
                    psv = ps_main.tile([128, 384], F32, tag="ps")
                    for k in range(KT):
                        nc.tensor.matmul(
                            psv[:, :],
                            xt_sb[:, k * NT + t * 128 : k * NT + (t + 1) * 128],
                            wq_sb[:, k * C3 + 1536 + j * 384 : k * C3 + 1536 + (j + 1) * 384],
                            start=(k == 0),
                            stop=(k == KT - 1),
                        )
                    for hh in range(6):
                        h = 6 * j + hh
                        nc.vector.tensor_copy(
                            vaug[:, t * H * VA + h * VA : t * H * VA + h * VA + HD],
                            psv[:, hh * HD : (hh + 1) * HD],
                        )

            # qkT: [1536, 1024] = w_qkv[:, :1536].T @ xT
            for m in range(MQK):
                for n in range(NQ):
                    psq = ps_main.tile([128, 512], F32, tag="ps")
                    for k in range(KT):
                        nc.tensor.matmul(
                            psq[:, :],
                            wq_sb[:, k * C3 + m * 128 : k * C3 + (m + 1) * 128],
                            xt_sb[:, k * NT + n * 512 : k * NT + (n + 1) * 512],
                            start=(k == 0),
                            stop=(k == KT - 1),
                        )
                    nc.vector.tensor_copy(
                        qk_sb[:, m * NT + n * 512 : m * NT + n * 512 + 512], psq[:, :]
                    )

        # ---------------- phase 2: attention, software-pipelined ----------------
        attn = stack.enter_context(tc.tile_pool(name="attn", bufs=1))
        wp_sb = attn.tile([128, KT * C], F32R)
        bp_sb = attn.tile([128, KT], F32)
        for k in range(KT):
            nc.sync.dma_start(
                out=wp_sb[:, k * C : (k + 1) * C],
                in_=wproj_d[k * 128 : (k + 1) * 128, :],
            )
        nc.sync.dma_start(out=bp_sb[:, :], in_=bproj_d[:, :])

        expst_tiles = {}
        pv_state = {}

        def emit_st(h):
            p0 = (h % 2) * 64
            qblk = h // 2
            kblk = 6 + h // 2
            et = attn.tile([128, TT * NT], BF16, name=f"expst{h}",
                           tag="expst", bufs=2)
            expst_tiles[h] = et
            for kt in range(TT):
                for qs in range(NQ):
                    pss = ps_main.tile([128, 512], F32, tag="ps")
                    nc.tensor.matmul(
                        pss[:, :],
                        qk_sb[p0 : p0 + 64,
                                  kblk * NT + kt * 128 : kblk * NT + (kt + 1) * 128],
                        qk_sb[p0 : p0 + 64,
                                  qblk * NT + qs * 512 : qblk * NT + (qs + 1) * 512],
                        start=True,
                        stop=True,
                    )
                    nc.scalar.activation(
                        et[:, kt * NT + qs * 512 : kt * NT + qs * 512 + 512],
                        pss[:, :],
                        mybir.ActivationFunctionType.Exp,
                        scale=SCALE,
                    )

        def emit_pv(h):
            et = expst_tiles.pop(h)
            psos = []
            for qs in range(NQ):
                pso = ps_pv.tile([VA, 512], F32, name=f"pso{h}_{qs}",
                                 tag="pso", bufs=4)
                for kt in range(TT):
                    nc.tensor.matmul(
                        pso[:, :],
                        vaug[:, kt * H * VA + h * VA : kt * H * VA + (h + 1) * VA],
                        et[:, kt * NT + qs * 512 : kt * NT + qs * 512 + 512],
                        start=(kt == 0),
                        stop=(kt == TT - 1),
                    )
                psos.append(pso)
            drow = attn.tile([1, NT], F32, name=f"drow{h}", tag="drow", bufs=3)
            for qs in range(NQ):
                nc.vector.tensor_copy(
                    drow[0:1, qs * 512 : qs * 512 + 512], psos[qs][64:65, :]
                )
            rrow = attn.tile([1, NT], F32, name=f"rrow{h}", tag="rrow", bufs=3)
            nc.vector.reciprocal(rrow[0:1, :], drow[0:1, :])
            pv_state[h] = (psos, rrow)

        def emit_norm(h):
            p0 = (h % 2) * 64
            qblk = h // 2
            psos, rrow = pv_state.pop(h)
            for qs in range(NQ):
                rbc = attn.tile([64, 512], F32, name=f"rbc{h}_{qs}",
                                tag="rbc", bufs=2)
                nc.gpsimd.partition_broadcast(rbc[:, :], rrow[0:1, qs * 512 : qs * 512 + 512])
                nc.vector.tensor_mul(
                    ot[p0 : p0 + 64,
                       qblk * NT + qs * 512 : qblk * NT + qs * 512 + 512],
                    psos[qs][0:64, :],
                    rbc[:, :],
                )

        # pipeline: PE order = ST_0, ST_1, PV_0, [ST_h+2, PV_h+1, NORM_h] ...
        emit_st(0)
        emit_st(1)
        emit_pv(0)
        for h in range(2, H):
            emit_st(h)
            emit_pv(h - 1)
            emit_norm(h - 2)
        emit_pv(H - 1)
        emit_norm(H - 2)
        emit_norm(H - 1)

        # ---------------- phase 3: output projection ----------------
        for m in range(KT):
            for ns in range(NQ):
                psy = ps_main.tile([128, 512], F32, tag="ps")
                for k in range(KT):
                    nc.tensor.matmul(
                        psy[:, :],
                        wp_sb[:, k * C + m * 128 : k * C + (m + 1) * 128],
                        ot[:, k * NT + ns * 512 : k * NT + (ns + 1) * 512],
                        start=(k == 0),
                        stop=(k == KT - 1),
                    )
                yt = attn.tile([128, 512], F32, name=f"yt{m}_{ns}", tag="yt", bufs=3)
                nc.vector.tensor_scalar_add(yt[:, :], psy[:, :], bp_sb[:, m : m + 1])
                nc.sync.dma_start(
                    out=out_d[m * 128 : (m + 1) * 128, ns * 512 : (ns + 1) * 512],
                    in_=yt[:, :],
                )


_NC = None


def build_nc():
    global _NC
    if _NC is None:
        nc = bacc.Bacc(
            trn_type="TRN2",
            target_bir_lowering=False,
            debug=False,
            enable_asserts=False,
            num_devices=8,
        )
        with tile.TileContext(nc) as tc:
            build_graph(tc)
        nc.compile()
        _NC = nc
    return _NC


def make_in_maps(x, w_qkv, w_proj, b_proj):
    x = np.ascontiguousarray(np.asarray(x, dtype=np.float32))
    w_qkv = np.ascontiguousarray(np.asarray(w_qkv, dtype=np.float32))
    w_proj = np.ascontiguousarray(np.asarray(w_proj, dtype=np.float32))
    b_proj = np.asarray(b_proj, dtype=np.float32)
    xT = np.ascontiguousarray(x.transpose(0, 2, 1))           # [8, 768, 1024]
    bp = np.ascontiguousarray(b_proj.reshape(KT, 128).T)      # [128, 6]
    return [
        {"xT": xT[i], "wqkv": w_qkv, "wproj": w_proj, "bproj": bp}
        for i in range(B)
    ]


def run_on_hw(in_maps, trace=False, **kwargs):
    from concourse.bass_utils import run_bass_kernel_spmd

    nc = build_nc()
    return run_bass_kernel_spmd(
        nc, in_maps, core_ids=list(range(B)), trace=trace, **kwargs
    )


def kernel(x, w_qkv, w_proj, b_proj):
    in_maps = make_in_maps(x, w_qkv, w_proj, b_proj)
    res = run_on_hw(in_maps, trace=False)
    out = np.stack([np.asarray(res.results[i]["out"]).T for i in range(B)])
    return np.ascontiguousarray(out.astype(np.float32))


# revision 14
# speedup vs baseline: 1.1621x; 1.1621x over previous
"""Multi-head attention (B=8, N=1024, C=768, H=12) on 8 TRN2 NeuronCores.

Sharding: data-parallel over the batch — core i computes batch element i.
No collectives.

Per-core math (all feature-major to avoid on-device transposes):
  qkT   = w_qkv[:, :1536].T @ xT            # [1536, 1024]  (q rows 0:768, k rows 768:1536)
  v_tok = xT.T @ w_qkv[:, 1536:]            # [1024, 768]   token-major, + ones col per head
  per head h:
    ST   = kT_h.T(as lhsT) ... = k @ q^T    # [1024k, 1024q]  (scores transposed)
    E    = exp(SCALE * ST)                  # bf16, no max-subtraction (scores ~ N(0,1))
    [O_un; d] = [v_h | 1].T @ E             # [65, 1024q]: rows 0:64 = (P@V)^T un-normalized,
                                            #              row 64 = softmax denominator
    OT_h = O_un * (1/d broadcast)           # broadcast via K=1 outer-product matmul
  yT = w_proj.T @ OT + b_proj               # [768, 1024]

Host side: kernel() takes full inputs, pre-transposes x, runs SPMD on 8 cores,
re-transposes/stacks the per-core outputs.
"""

import os
import sys

import numpy as np

for _p in ("/opt/trn_rl_repo", "/root/.axon_site/_ro/trn_rl_repo"):
    if os.path.isdir(_p) and _p not in sys.path:
        sys.path.insert(0, _p)

import concourse.bacc as bacc
import concourse.mybir as mybir
import concourse.tile as tile

F32 = mybir.dt.float32
F32R = mybir.dt.float32r
BF16 = mybir.dt.bfloat16

B, NT, C = 8, 1024, 768
H, HD = 12, 64
C3 = 3 * C          # 2304
SCALE = HD ** -0.5  # 0.125
KT = C // 128       # 6   k-tiles over the C contraction
MQK = 1536 // 128   # 12  row-blocks of qkT
TT = NT // 128      # 8   token tiles
NQ = NT // 512      # 2   512-wide q slices
VA = HD + 1         # 65  v columns per head + ones column


def build_graph(tc):
    nc = tc.nc
    xt_d = nc.dram_tensor("xT", [C, NT], F32R, kind="ExternalInput").ap()
    wqkv_d = nc.dram_tensor("wqkv", [C, C3], F32R, kind="ExternalInput").ap()
    wproj_d = nc.dram_tensor("wproj", [C, C], F32R, kind="ExternalInput").ap()
    bproj_d = nc.dram_tensor("bproj", [128, KT], F32, kind="ExternalInput").ap()
    out_d = nc.dram_tensor("out", [C, NT], F32, kind="ExternalOutput").ap()

    from contextlib import ExitStack

    with ExitStack() as stack:
        persist = stack.enter_context(tc.tile_pool(name="persist", bufs=1))
        qk_sb = persist.tile([128, MQK * NT], BF16)       # qkT feature-major (bf16)
        vaug = persist.tile([128, TT * H * VA], BF16)    # [v_h | 1] per head, token-major
        ot = persist.tile([128, KT * NT], F32R)           # attention out, feature-major

        ps_main = stack.enter_context(
            tc.tile_pool(name="ps_main", bufs=4, space="PSUM")
        )
        ps_pv = stack.enter_context(tc.tile_pool(name="ps_pv", bufs=4, space="PSUM"))

        nc.vector.memset(vaug[:, :], 1.0)

        # ---------------- phase 1: qkT and v_tok ----------------
        with tc.tile_pool(name="ph1", bufs=1) as ph1:
            wq_sb = ph1.tile([128, KT * C3], F32R)
            xt_sb = ph1.tile([128, KT * NT], F32R)

            for k in range(KT):
                nc.sync.dma_start(
                    out=xt_sb[:, k * NT : (k + 1) * NT],
                    in_=xt_d[k * 128 : (k + 1) * 128, :],
                )
                # v columns first so v_tok matmuls can start early
                nc.sync.dma_start(
                    out=wq_sb[:, k * C3 + 1536 : k * C3 + C3],
                    in_=wqkv_d[k * 128 : (k + 1) * 128, 1536:C3],
                )
            for k in range(KT):
                nc.sync.dma_start(
                    out=wq_sb[:, k * C3 : k * C3 + 1536],
                    in_=wqkv_d[k * 128 : (k + 1) * 128, 0:1536],
                )

            # v_tok: [1024 tok, 768] = xT.T @ w_qkv[:, 1536:2304], evicted into
            # vaug (bf16) with a ones column per head
            for t in range(TT):
                for j in range(2):  # 384-wide v-col slices → heads 6j..6j+5
                    psv = ps_main.tile([128, 384], F32, tag="ps")
                    for k in range(KT):
                        nc.tensor.matmul(
                            psv[:, :],
                            xt_sb[:, k * NT + t * 128 : k * NT + (t + 1) * 128],
                            wq_sb[:, k * C3 + 1536 + j * 384 : k * C3 + 1536 + (j + 1) * 384],
                            start=(k == 0),
                            stop=(k == KT - 1),
                        )
                    for hh in range(6):
                        h = 6 * j + hh
                        nc.vector.tensor_copy(
                            vaug[:, t * H * VA + h * VA : t * H * VA + h * VA + HD],
                            psv[:, hh * HD : (hh + 1) * HD],
                        )

            # qkT: [1536, 1024] = w_qkv[:, :1536].T @ xT
            for m in range(MQK):
                for n in range(NQ):
                    psq = ps_main.tile([128, 512], F32, tag="ps")
                    for k in range(KT):
                        nc.tensor.matmul(
                            psq[:, :],
                            wq_sb[:, k * C3 + m * 128 : k * C3 + (m + 1) * 128],
                            xt_sb[:, k * NT + n * 512 : k * NT + (n + 1) * 512],
                            start=(k == 0),
                            stop=(k == KT - 1),
                        )
                    nc.vector.tensor_copy(
                        qk_sb[:, m * NT + n * 512 : m * NT + n * 512 + 512], psq[:, :]
                    )

        # ---------------- phase 2: attention, software-pipelined ----------------
        attn = stack.enter_context(tc.tile_pool(name="attn", bufs=1))
        wp_sb = attn.tile([128, KT * C], F32R)
        bp_sb = attn.tile([128, KT], F32)
        for k in range(KT):
            nc.sync.dma_start(
                out=wp_sb[:, k * C : (k + 1) * C],
                in_=wproj_d[k * 128 : (k + 1) * 128, :],
            )
        nc.sync.dma_start(out=bp_sb[:, :], in_=bproj_d[:, :])

        expst_tiles = {}
        pv_state = {}

        def emit_st(h):
            p0 = (h % 2) * 64
            qblk = h // 2
            kblk = 6 + h // 2
            et = attn.tile([128, TT * NT], BF16, name=f"expst{h}",
                           tag="expst", bufs=2)
            expst_tiles[h] = et
            for kt in range(TT):
                for qs in range(NQ):
                    pss = ps_main.tile([128, 512], F32, tag="ps")
                    nc.tensor.matmul(
                        pss[:, :],
                        qk_sb[p0 : p0 + 64,
                                  kblk * NT + kt * 128 : kblk * NT + (kt + 1) * 128],
                        qk_sb[p0 : p0 + 64,
                                  qblk * NT + qs * 512 : qblk * NT + (qs + 1) * 512],
                        start=True,
                        stop=True,
                    )
                    nc.scalar.activation(
                        et[:, kt * NT + qs * 512 : kt * NT + qs * 512 + 512],
                        pss[:, :],
                        mybir.ActivationFunctionType.Exp,
                        scale=SCALE,
                    )

        dtiles = {}

        def emit_pv(h):
            et = expst_tiles.pop(h)
            p0 = (h % 2) * 64
            qblk = h // 2
            if h % 4 == 0:
                dtiles[h // 4] = attn.tile([97, NT], F32, name=f"d4_{h//4}",
                                           tag="d4", bufs=2)
                nc.vector.memset(dtiles[h // 4][:, :], 1.0)
            d6 = dtiles[h // 4]
            for qs in range(NQ):
                pso = ps_pv.tile([VA, 512], F32, name=f"pso{h}_{qs}",
                                 tag="pso", bufs=4)
                for kt in range(TT):
                    nc.tensor.matmul(
                        pso[:, :],
                        vaug[:, kt * H * VA + h * VA : kt * H * VA + (h + 1) * VA],
                        et[:, kt * NT + qs * 512 : kt * NT + qs * 512 + 512],
                        start=(kt == 0),
                        stop=(kt == TT - 1),
                    )
                # stash denominator row; evict unnormalized O_un^T
                dp = 32 * (h % 4)
                nc.vector.tensor_copy(
                    d6[dp : dp + 1, qs * 512 : qs * 512 + 512],
                    pso[64:65, :],
                )
                nc.vector.tensor_copy(
                    ot[p0 : p0 + 64,
                       qblk * NT + qs * 512 : qblk * NT + qs * 512 + 512],
                    pso[0:64, :],
                )

        def emit_norm_batch(b):
            d4 = dtiles.pop(b)
            r4 = attn.tile([97, NT], F32, name=f"r4_{b}", tag="r4", bufs=2)
            nc.vector.reciprocal(r4[:, :], d4[:, :])
            for hh in range(4):
                h = 4 * b + hh
                p0 = (h % 2) * 64
                qblk = h // 2
                if hh == 0:
                    rsrc = r4
                else:
                    rsrc = attn.tile([1, NT], F32, name=f"r0_{h}", tag="r0", bufs=3)
                    nc.vector.tensor_copy(rsrc[0:1, :], r4[32 * hh : 32 * hh + 1, :])
                for qs in range(NQ):
                    rbc = attn.tile([128, 512], F32, name=f"rbc{h}_{qs}",
                                    tag="rbc", bufs=3)
                    nc.gpsimd.partition_broadcast(
                        rbc[:, :], rsrc[0:1, qs * 512 : qs * 512 + 512]
                    )
                    rsl = rbc[p0 : p0 + 64, :]
                    osl = ot[p0 : p0 + 64,
                             qblk * NT + qs * 512 : qblk * NT + qs * 512 + 512]
                    nc.vector.tensor_mul(osl, osl, rsl)

        # pipeline: PE order = ST_0, ST_1, [ST_h+2, PV_h+1] ...; norm per 6-head batch
        emit_st(0)
        emit_st(1)
        emit_pv(0)
        for h in range(2, H):
            emit_st(h)
            emit_pv(h - 1)
            if (h - 1) % 4 == 3 and h < H:
                emit_norm_batch((h - 1) // 4)
        emit_pv(H - 1)
        emit_norm_batch(2)

        # ---------------- phase 3: output projection ----------------
        for m in range(KT):
            for ns in range(NQ):
                psy = ps_main.tile([128, 512], F32, tag="ps")
                for k in range(KT):
                    nc.tensor.matmul(
                        psy[:, :],
                        wp_sb[:, k * C + m * 128 : k * C + (m + 1) * 128],
                        ot[:, k * NT + ns * 512 : k * NT + (ns + 1) * 512],
                        start=(k == 0),
                        stop=(k == KT - 1),
                    )
                yt = attn.tile([128, 512], F32, name=f"yt{m}_{ns}", tag="yt", bufs=3)
                nc.vector.tensor_scalar_add(yt[:, :], psy[:, :], bp_sb[:, m : m + 1])
                nc.sync.dma_start(
                    out=out_d[m * 128 : (m + 1) * 128, ns * 512 : (ns + 1) * 512],
                    in_=yt[:, :],
                )


_NC = None


def build_nc():
    global _NC
    if _NC is None:
        nc = bacc.Bacc(
            trn_type="TRN2",
            target_bir_lowering=False,
            debug=False,
            enable_asserts=False,
            num_devices=8,
        )
        with tile.TileContext(nc) as tc:
            build_graph(tc)
        nc.compile()
        _NC = nc
    return _NC


def make_in_maps(x, w_qkv, w_proj, b_proj):
    x = np.ascontiguousarray(np.asarray(x, dtype=np.float32))
    w_qkv = np.ascontiguousarray(np.asarray(w_qkv, dtype=np.float32))
    w_proj = np.ascontiguousarray(np.asarray(w_proj, dtype=np.float32))
    b_proj = np.asarray(b_proj, dtype=np.float32)
    xT = np.ascontiguousarray(x.transpose(0, 2, 1))           # [8, 768, 1024]
    bp = np.ascontiguousarray(b_proj.reshape(KT, 128).T)      # [128, 6]
    return [
        {"xT": xT[i], "wqkv": w_qkv, "wproj": w_proj, "bproj": bp}
        for i in range(B)
    ]


def run_on_hw(in_maps, trace=False, **kwargs):
    from concourse.bass_utils import run_bass_kernel_spmd

    nc = build_nc()
    return run_bass_kernel_spmd(
        nc, in_maps, core_ids=list(range(B)), trace=trace, **kwargs
    )


def kernel(x, w_qkv, w_proj, b_proj):
    in_maps = make_in_maps(x, w_qkv, w_proj, b_proj)
    res = run_on_hw(in_maps, trace=False)
    out = np.stack([np.asarray(res.results[i]["out"]).T for i in range(B)])
    return np.ascontiguousarray(out.astype(np.float32))
